# revision 1
# baseline (speedup 1.0000x reference)
"""Trainium2 Bass kernel for nn_MultiHeadAttention_61357902791348.

Sharding: 12 heads on 8 cores. Core pair (2p, 2p+1) owns heads {3p, 3p+1}
fully and splits head 3p+2's query rows (even core: rows [0,1600), odd:
[1600,3200)) -- balanced head/sequence-hybrid tensor parallelism with no
device collectives. Each core emits partial out-projection results
(y_own for its full head over all rows, y_sh for its half of the shared
head); the host sums the 8 partials and adds bo.

Per-core math (all matmuls in fp32r / TF32 on the PE):
  Q^T/K^T/V^T = W.T @ x^T          (HD-major, accumulate 12 D-chunks)
  RMS-norm folded: rs = rsqrt(sumsq +/- eps scaling); q_scale/k_scale and
  the rotation-pair swap folded into host-precomputed cos/sin tables.
  RoPE = q * C + (R @ q) * S       (R = constant pair-rotation matrix)
  scores^T[j,i] = K^T.T @ Q^T      (j=key token on partitions)
  exp via ACT with per-partition scale = rs_k[j]; 1/sqrt(HD) and rs_q
  folded into Q.
  sums[i] = ones.T @ exp           (PE ones-matmul, accumulated over j)
  ao^T[hd,i] = V_tok.T @ exp       (V transposed to token-major via PE)
  ao^T *= bcast(1/sums)            (K=1 ones-matmul broadcast)
  y = ao^T.T @ Wo_rows             (partial out-projection)
"""

import numpy as np

B, N, D = 1, 3200, 1536
NH, HD = 12, 128
F, Hg, Wg = 8, 20, 20
EPS = 1e-6
NS = 1600          # shared-head query rows per core
NCH = D // 128     # 12 D-chunks
PW = 400           # projection/rope moving tile width
NQ = N + NS        # 4800 q tokens per core (own + shared)
NK = 2 * N         # 6400 k tokens per core (own + shared heads)

_CACHE = {}


def _build():
    import concourse.bacc as bacc
    import concourse.mybir as mybir
    import concourse.tile as tile

    F32 = mybir.dt.float32
    F32R = mybir.dt.float32r
    AF = mybir.ActivationFunctionType

    nc = bacc.Bacc("TRN2", target_bir_lowering=False, debug=False)

    xT = nc.dram_tensor("xT", [D, N], F32R, kind="ExternalInput")
    w6 = nc.dram_tensor("w6", [6, D, HD], F32R, kind="ExternalInput")
    bias6 = nc.dram_tensor("bias6", [HD, 6], F32, kind="ExternalInput")
    cq = nc.dram_tensor("cq", [HD, N], F32, kind="ExternalInput")
    sq = nc.dram_tensor("sq", [HD, N], F32, kind="ExternalInput")
    ck = nc.dram_tensor("ck", [HD, N], F32, kind="ExternalInput")
    sk = nc.dram_tensor("sk", [HD, N], F32, kind="ExternalInput")
    rotm = nc.dram_tensor("rotm", [HD, HD], F32R, kind="ExternalInput")
    eye = nc.dram_tensor("eye", [HD, HD], F32R, kind="ExternalInput")
    ones2d = nc.dram_tensor("ones2d", [HD, HD], F32R, kind="ExternalInput")
    wo2 = nc.dram_tensor("wo2", [2, HD, D], F32R, kind="ExternalInput")
    y_own = nc.dram_tensor("y_own", [N, D], F32, kind="ExternalOutput")
    y_sh = nc.dram_tensor("y_sh", [NS, D], F32, kind="ExternalOutput")

    def subtiles(total, width):
        return [(o, min(width, total - o)) for o in range(0, total, width)]

    with tile.TileContext(nc) as tc:
        import contextlib

        stack = contextlib.ExitStack()
        with stack:
            persist = stack.enter_context(tc.tile_pool(name="persist", bufs=1))
            qT = persist.tile([128, NQ], F32R, tag="qT")
            kT = persist.tile([128, NK], F32R, tag="kT")
            vtok = persist.tile([128, 2, 25, HD], F32R, tag="vtok")
            bias_sb = persist.tile([HD, 6], F32, tag="bias")
            nc.sync.dma_start(bias_sb[:], bias6[:])
            ones_sb = persist.tile([HD, HD], F32R, tag="ones")
            nc.sync.dma_start(ones_sb[:], ones2d[:])
            rot_sb = persist.tile([HD, HD], F32R, tag="rot")
            nc.sync.dma_start(rot_sb[:], rotm[:])
            eye_sb = persist.tile([HD, HD], F32R, tag="eye")
            nc.sync.dma_start(eye_sb[:], eye[:])
            rsk_sb = persist.tile([128, 50], F32, tag="rsk")
            bias_q = persist.tile([1, 1], F32, tag="bias_q")
            nc.vector.memset(bias_q[:], HD * EPS)
            bias_k = persist.tile([1, 1], F32, tag="bias_k")
            nc.vector.memset(bias_k[:], EPS)

            # ---------------- projection phase ----------------
            # vT holds V^T before token-major transposition
            with tc.tile_pool(name="vt", bufs=1) as vt_pool:
                vT = vt_pool.tile([128, NK], F32R, tag="vT")
                with tc.tile_pool(name="xt", bufs=1) as xt_pool, \
                     tc.tile_pool(name="wld", bufs=4) as w_pool, \
                     tc.tile_pool(name="pp", bufs=4, space="PSUM") as pp:
                    for half in range(2):
                        h0 = half * 1600
                        xt = xt_pool.tile([128, NCH, 1600], F32R, tag="xt")
                        for c in range(NCH):
                            nc.sync.dma_start(
                                xt[:, c, :], xT[c * 128:(c + 1) * 128, h0:h0 + 1600]
                            )
                        # blocks: 0 q_own, 1 q_sh, 2 k_own, 3 k_sh, 4 v_own, 5 v_sh
                        for b in range(6):
                            if b == 1 and half == 1:
                                continue  # shared-head q only needs tokens [0,1600)
                            if b == 0:
                                dst, d0 = qT, h0
                            elif b == 1:
                                dst, d0 = qT, N + h0
                            elif b in (2, 3):
                                dst, d0 = kT, (b - 2) * N + h0
                            else:
                                dst, d0 = vT, (b - 4) * N + h0
                            wtiles = []
                            for c in range(NCH):
                                wt = w_pool.tile([128, HD], F32R, tag="w")
                                nc.sync.dma_start(
                                    wt[:], w6[b, c * 128:(c + 1) * 128, :]
                                )
                                wtiles.append(wt)
                            for (o, w) in subtiles(1600, PW):
                                ps = pp.tile([128, PW], F32, tag="pp")
                                for c in range(NCH):
                                    nc.tensor.matmul(
                                        ps[:, :w], wtiles[c][:], xt[:, c, o:o + w],
                                        start=(c == 0), stop=(c == NCH - 1),
                                    )
                                nc.vector.tensor_scalar_add(
                                    dst[:, d0 + o:d0 + o + w], ps[:, :w],
                                    bias_sb[:, b:b + 1],
                                )

                # ---------------- rope + rms-norm phase ----------------
                RW = 512
                with tc.tile_pool(name="cs", bufs=3) as cs_pool, \
                     tc.tile_pool(name="rtmp", bufs=3) as rtmp, \
                     tc.tile_pool(name="rps", bufs=2, space="PSUM") as rps, \
                     tc.tile_pool(name="sps", bufs=2, space="PSUM") as sps:
                    rope_work = []
                    for kind, seg0, seglen in (
                        ("q", 0, N), ("q", N, NS), ("k", 0, N), ("k", N, N),
                    ):
                        for (ol, w) in subtiles(seglen, RW):
                            rope_work.append((kind, seg0 + ol, ol, w))
                    for (kind, o, tok, w) in rope_work:
                        if True:
                            big = qT if kind == "q" else kT
                            cdr, sdr = (cq, sq) if kind == "q" else (ck, sk)
                            ct = cs_pool.tile([128, RW], F32, tag="c")
                            st = cs_pool.tile([128, RW], F32, tag="s")
                            nc.sync.dma_start(ct[:, :w], cdr[:, tok:tok + w])
                            nc.sync.dma_start(st[:, :w], sdr[:, tok:tok + w])
                            src = big[:, o:o + w]
                            # sumsq via ones-matmul on q^2
                            q2 = rtmp.tile([128, RW], F32R, tag="q2")
                            nc.vector.tensor_mul(q2[:, :w], src, src)
                            ssq = sps.tile([1, RW], F32, tag="ssq")
                            nc.tensor.matmul(
                                ssq[:, :w], ones_sb[:, 0:1], q2[:, :w],
                                start=True, stop=True,
                            )
                            # rs row: q: rsqrt(ssq + HD*eps) (incl. 1/sqrt(HD));
                            #         k: rsqrt(ssq/HD + eps)
                            sqv = rtmp.tile([1, RW], F32, tag="sqv")
                            if kind == "q":
                                nc.scalar.activation(
                                    sqv[:, :w], ssq[:, :w], AF.Sqrt,
                                    bias=bias_q[:], scale=1.0,
                                )
                            else:
                                nc.scalar.activation(
                                    sqv[:, :w], ssq[:, :w], AF.Sqrt,
                                    bias=bias_k[:], scale=1.0 / HD,
                                )
                            rsv = rtmp.tile(
                                [1, RW], F32R,
                                tag="rsv", name=f"rsv_{kind}_{o}",
                            )
                            with nc.allow_low_precision(reason="tf32 rs rows"):
                                nc.vector.reciprocal(rsv[:, :w], sqv[:, :w])
                            if kind == "k":
                                # transpose rs_k row chunks onto partitions via
                                # K=1 matmuls (N=2 for the fp32r 8-byte dst rule)
                                rsp = sps.tile([128, 8], F32, tag="ssq",
                                               name=f"rsp_{o}")
                                nch = w // 128
                                for cc in range(nch):
                                    nc.tensor.matmul(
                                        rsp[:, 2 * cc:2 * cc + 2],
                                        rsv[0:1, cc * 128:(cc + 1) * 128],
                                        ones_sb[0:1, 0:2],
                                        start=True, stop=True,
                                    )
                                nc.vector.tensor_copy(
                                    rsk_sb[:, o // 128:o // 128 + nch],
                                    rsp[:, 0:2 * nch:2],
                                )
                            # rotate pairs via PE
                            rot = rps.tile([128, RW], F32, tag="rot")
                            nc.tensor.matmul(
                                rot[:, :w], rot_sb[:], src, start=True, stop=True,
                            )
                            m1 = rtmp.tile([128, RW], F32, tag="m1")
                            nc.vector.tensor_mul(m1[:, :w], src, ct[:, :w])
                            m2 = rtmp.tile([128, RW], F32, tag="m2")
                            nc.vector.tensor_mul(m2[:, :w], rot[:, :w], st[:, :w])
                            if kind == "k":
                                nc.vector.tensor_add(src, m1[:, :w], m2[:, :w])
                            else:
                                qr = rtmp.tile([128, RW], F32, tag="qr")
                                nc.vector.tensor_add(qr[:, :w], m1[:, :w], m2[:, :w])
                                # apply rs_q via K=1 broadcast matmul
                                bc = rps.tile([128, RW], F32, tag="bc")
                                nc.tensor.matmul(
                                    bc[:, :w], ones_sb[0:1, :], rsv[:, :w],
                                    start=True, stop=True,
                                )
                                nc.vector.tensor_mul(src, qr[:, :w], bc[:, :w])

                # V^T -> token-major V via PE transpose
                with tc.tile_pool(name="tps", bufs=3, space="PSUM") as tps:
                    for h in range(2):
                        for jt in range(25):
                            tp = tps.tile([128, HD], F32R, tag="tp")
                            nc.tensor.transpose(
                                tp[:], vT[:, h * N + jt * 128:h * N + (jt + 1) * 128],
                                eye_sb[:],
                            )
                            nc.vector.tensor_copy(vtok[:, h, jt, :], tp[:])

            # ---------------- attention phase ----------------
            with tc.tile_pool(name="scps", bufs=2, space="PSUM") as scps, \
                 tc.tile_pool(name="aops", bufs=1, space="PSUM") as aops, \
                 tc.tile_pool(name="smps", bufs=2, space="PSUM") as smps, \
                 tc.tile_pool(name="expp", bufs=3) as expp, \
                 tc.tile_pool(name="atmp", bufs=4) as atmp:
                aoT = persist.tile([128, NQ], F32R, tag="aoT")
                for unit in range(2):
                    q0, qw, head = (0, N, 0) if unit == 0 else (N, NS, 1)
                    for (co, cw) in subtiles(qw, 1024):
                        subs = subtiles(cw, 512)
                        ao = aops.tile([128, 1024], F32, tag="ao")
                        sm = [smps.tile([1, 512], F32, tag="sm", name=f"sm{unit}_{co}_{i}") for i, _ in enumerate(subs)]
                        for jt in range(25):
                            gjt = head * 25 + jt
                            kap = kT[:, gjt * 128:(gjt + 1) * 128]
                            sc = scps.tile([128, 1024], F32, tag="sc")
                            for (so, sw) in subs:
                                nc.tensor.matmul(
                                    sc[:, so:so + sw], kap,
                                    qT[:, q0 + co + so:q0 + co + so + sw],
                                    start=True, stop=True,
                                )
                            ex = expp.tile([128, 1024], F32R, tag="ex")
                            nc.scalar.activation(
                                ex[:, :cw], sc[:, :cw], AF.Exp,
                                scale=rsk_sb[:, gjt:gjt + 1],
                            )
                            for si, (so, sw) in enumerate(subs):
                                nc.tensor.matmul(
                                    sm[si][:, :sw], ones_sb[:, 0:1],
                                    ex[:, so:so + sw],
                                    start=(jt == 0), stop=(jt == 24),
                                )
                                nc.tensor.matmul(
                                    ao[:, so:so + sw], vtok[:, head, jt, :],
                                    ex[:, so:so + sw],
                                    start=(jt == 0), stop=(jt == 24),
                                )
                        for si, (so, sw) in enumerate(subs):
                            inv = atmp.tile([1, 512], F32R, tag="inv")
                            with nc.allow_low_precision(reason="tf32 inv sums"):
                                nc.vector.reciprocal(inv[:, :sw], sm[si][:, :sw])
                            bc = scps.tile([128, 1024], F32, tag="sc")
                            nc.tensor.matmul(
                                bc[:, :sw], ones_sb[0:1, :], inv[:, :sw],
                                start=True, stop=True,
                            )
                            bcs = atmp.tile([128, 512], F32, tag="bcs")
                            nc.vector.tensor_copy(bcs[:, :sw], bc[:, :sw])
                            nc.vector.tensor_mul(
                                aoT[:, q0 + co + so:q0 + co + so + sw],
                                ao[:, so:so + sw], bcs[:, :sw],
                            )

            # ---------------- out-projection phase ----------------
            with tc.tile_pool(name="wo", bufs=1) as wo_pool, \
                 tc.tile_pool(name="yout", bufs=3) as yout, \
                 tc.tile_pool(name="ops", bufs=4, space="PSUM") as ops:
                wo_sb = [wo_pool.tile([HD, D], F32R, tag=f"wo{u}", name=f"wo_sb{u}") for u in range(2)]
                for u in range(2):
                    nc.sync.dma_start(wo_sb[u][:], wo2[u, :, :])
                for unit in range(2):
                    q0, qw = (0, N) if unit == 0 else (N, NS)
                    ydst = y_own if unit == 0 else y_sh
                    for (it, iw) in subtiles(qw, 128):
                        yt = yout.tile([128, D], F32, tag="yt")
                        for ct in range(3):
                            op = ops.tile([128, 512], F32, tag="op")
                            nc.tensor.matmul(
                                op[:iw, :], aoT[:, q0 + it:q0 + it + iw],
                                wo_sb[unit][:, ct * 512:(ct + 1) * 512],
                                start=True, stop=True,
                            )
                            if ct == 2:
                                nc.vector.tensor_copy(
                                    yt[:iw, ct * 512:(ct + 1) * 512], op[:iw, :]
                                )
                            else:
                                nc.scalar.activation(
                                    yt[:iw, ct * 512:(ct + 1) * 512], op[:iw, :],
                                    AF.Copy, bias=0.0, scale=1.0,
                                )
                        nc.sync.dma_start(ydst[it:it + iw, :], yt[:iw, :])

    nc.compile()
    return nc


def _get_nc():
    if "nc" not in _CACHE:
        _CACHE["nc"] = _build()
    return _CACHE["nc"]


def _host_prep(inputs):
    x = np.asarray(inputs["x"], np.float32)[0]          # [N, D]
    Wq = np.asarray(inputs["Wq"], np.float32)
    Wk = np.asarray(inputs["Wk"], np.float32)
    Wv = np.asarray(inputs["Wv"], np.float32)
    Wo = np.asarray(inputs["Wo"], np.float32)
    bq = np.asarray(inputs["bq"], np.float32)
    bk = np.asarray(inputs["bk"], np.float32)
    bv = np.asarray(inputs["bv"], np.float32)
    qs = np.asarray(inputs["q_scale"], np.float32)
    ks = np.asarray(inputs["k_scale"], np.float32)
    ft = np.asarray(inputs["freqs_t"], np.float32)
    fh = np.asarray(inputs["freqs_h"], np.float32)
    fw = np.asarray(inputs["freqs_w"], np.float32)

    cos = np.zeros((N, HD // 2), np.float32)
    sin = np.zeros((N, HD // 2), np.float32)
    idx = np.arange(N)
    f_idx, h_idx, w_idx = idx // (Hg * Wg), (idx // Wg) % Hg, idx % Wg
    cos[:, 0:22], sin[:, 0:22] = ft[f_idx, :, 0], ft[f_idx, :, 1]
    cos[:, 22:43], sin[:, 22:43] = fh[h_idx, :, 0], fh[h_idx, :, 1]
    cos[:, 43:64], sin[:, 43:64] = fw[w_idx, :, 0], fw[w_idx, :, 1]
    C = np.repeat(cos, 2, axis=1).T.copy()               # [128, N]
    S = np.repeat(sin, 2, axis=1).T.copy()
    qs_sw = qs.reshape(64, 2)[:, ::-1].reshape(128)
    ks_sw = ks.reshape(64, 2)[:, ::-1].reshape(128)
    Cq, Sq = C * qs[:, None], S * qs_sw[:, None]
    Ck, Sk = C * ks[:, None], S * ks_sw[:, None]

    rotm = np.zeros((128, 128), np.float32)
    pr = np.arange(64)
    rotm[2 * pr + 1, 2 * pr] = -1.0
    rotm[2 * pr, 2 * pr + 1] = 1.0
    eye = np.eye(128, dtype=np.float32)
    ones2d = np.ones((128, 128), np.float32)

    xT = np.ascontiguousarray(x.T)                       # [D, N]
    perm_swap = np.concatenate([np.arange(1600, N), np.arange(0, 1600)])

    in_maps = []
    for core in range(8):
        pair, parity = core // 2, core % 2
        own, sh = 3 * pair + parity, 3 * pair + 2
        if parity == 0:
            xTc, Cqc, Sqc, Ckc, Skc = xT, Cq, Sq, Ck, Sk
        else:
            xTc = np.ascontiguousarray(xT[:, perm_swap])
            Cqc = np.ascontiguousarray(Cq[:, perm_swap])
            Sqc = np.ascontiguousarray(Sq[:, perm_swap])
            Ckc = np.ascontiguousarray(Ck[:, perm_swap])
            Skc = np.ascontiguousarray(Sk[:, perm_swap])
        w6 = np.stack([
            Wq[:, own * HD:(own + 1) * HD], Wq[:, sh * HD:(sh + 1) * HD],
            Wk[:, own * HD:(own + 1) * HD], Wk[:, sh * HD:(sh + 1) * HD],
            Wv[:, own * HD:(own + 1) * HD], Wv[:, sh * HD:(sh + 1) * HD],
        ])
        bias6 = np.stack([
            bq[own * HD:(own + 1) * HD], bq[sh * HD:(sh + 1) * HD],
            bk[own * HD:(own + 1) * HD], bk[sh * HD:(sh + 1) * HD],
            bv[own * HD:(own + 1) * HD], bv[sh * HD:(sh + 1) * HD],
        ], axis=1)
        wo2 = np.stack([
            Wo[own * HD:(own + 1) * HD, :], Wo[sh * HD:(sh + 1) * HD, :],
        ])
        in_maps.append({
            "xT": xTc, "w6": np.ascontiguousarray(w6),
            "bias6": np.ascontiguousarray(bias6),
            "cq": Cqc, "sq": Sqc, "ck": Ckc, "sk": Skc,
            "rotm": rotm, "eye": eye, "ones2d": ones2d,
            "wo2": np.ascontiguousarray(wo2),
        })
    return in_maps, perm_swap


def _gather(results, perm_swap, bo):
    inv_swap = perm_swap  # swapping halves is its own inverse
    y = np.zeros((N, D), np.float32)
    for core in range(8):
        parity = core % 2
        yo = results[core]["y_own"]
        ysh = results[core]["y_sh"]
        if parity == 0:
            y += yo
            y[0:1600] += ysh
        else:
            y += yo[inv_swap]
            y[1600:3200] += ysh
    y += bo[None, :]
    return y[None]


def run_internal(inputs, trace=False, **kw):
    from concourse.bass_utils import run_bass_kernel_spmd

    nc = _get_nc()
    in_maps, perm_swap = _host_prep(inputs)
    res = run_bass_kernel_spmd(
        nc, in_maps, core_ids=list(range(8)), trace=trace, **kw
    )
    bo = np.asarray(inputs["bo"], np.float32)
    y = _gather(res.results, perm_swap, bo)
    return y, res


def kernel(**inputs):
    y, _ = run_internal(inputs, trace=False)
    return y



# revision 13
# speedup vs baseline: 1.1535x; 1.1535x over previous
"""Trainium2 Bass kernel for nn_MultiHeadAttention_61357902791348.

Sharding: 12 heads on 8 cores. Core pair (2p, 2p+1) owns heads {3p, 3p+1}
fully and splits head 3p+2's query rows (even core: rows [0,1600), odd:
[1600,3200)) -- balanced head/sequence-hybrid tensor parallelism with no
device collectives. Each core emits partial out-projection results; the
host sums the 8 partials and adds bo.

v2 rewrite vs baseline:
  * all PE operands bf16 (moving-stream bytes halved; fp32 PSUM accum)
  * no [1,W] single-lane vector/scalar ops: RMS-norm and softmax-sum
    rows are transposed onto partitions (K=1 ones-matmul trick) before
    rsqrt/reciprocal
  * softmax normalization deferred to the out-projection PSUM->SBUF copy
    as a per-partition activation scale (tokens on partitions there)
  * attention software-pipelined: scores(jt+1) issued before sm/ao(jt)
    so the exp latency on ACT hides under PE work
  * rope(q chunk) -> attention(chunk) -> out-proj(chunk) interleaved in
    one loop so DVE rope work hides under attention PE work
"""

import numpy as np

B, N, D = 1, 3200, 1536
NH, HD = 12, 128
F, Hg, Wg = 8, 20, 20
EPS = 1e-6
NS = 1600          # shared-head query rows per core
NCH = D // 128     # 12 D-chunks
PW = 400           # projection moving tile width
RW = 512           # rope / attention chunk width
NQ = N + NS        # 4800 q tokens per core (own + shared)
NK = 2 * N         # 6400 k tokens per core (own + shared heads)
NCHUNK = 38        # ceil(NQ / 128) 128-token chunks (last is 64 wide)

_CACHE = {}


def subtiles(total, width):
    return [(o, min(width, total - o)) for o in range(0, total, width)]


def _build():
    import concourse.bacc as bacc
    import concourse.mybir as mybir
    import concourse.tile as tile

    F32 = mybir.dt.float32
    BF16 = mybir.dt.bfloat16
    AF = mybir.ActivationFunctionType

    nc = bacc.Bacc("TRN2", target_bir_lowering=False, debug=False)

    xT = nc.dram_tensor("xT", [D, N], BF16, kind="ExternalInput")
    w6 = nc.dram_tensor("w6", [6, D, HD], BF16, kind="ExternalInput")
    bias6 = nc.dram_tensor("bias6", [HD, 6], F32, kind="ExternalInput")
    cq = nc.dram_tensor("cq", [HD, N], BF16, kind="ExternalInput")
    sq = nc.dram_tensor("sq", [HD, N], BF16, kind="ExternalInput")
    ck = nc.dram_tensor("ck", [HD, N], BF16, kind="ExternalInput")
    sk = nc.dram_tensor("sk", [HD, N], BF16, kind="ExternalInput")
    rotm = nc.dram_tensor("rotm", [HD, HD], BF16, kind="ExternalInput")
    eye = nc.dram_tensor("eye", [HD, HD], BF16, kind="ExternalInput")
    ones2d = nc.dram_tensor("ones2d", [HD, HD], BF16, kind="ExternalInput")
    ones2df = nc.dram_tensor("ones2df", [HD, HD], F32, kind="ExternalInput")
    eyef = nc.dram_tensor("eyef", [HD, HD], F32, kind="ExternalInput")
    wo2 = nc.dram_tensor("wo2", [2, HD, D], BF16, kind="ExternalInput")
    y_own = nc.dram_tensor("y_own", [N, D], F32, kind="ExternalOutput")
    y_sh = nc.dram_tensor("y_sh", [NS, D], F32, kind="ExternalOutput")

    with tile.TileContext(nc) as tc:
        import contextlib

        stack = contextlib.ExitStack()
        with stack:
            persist = stack.enter_context(tc.tile_pool(name="persist", bufs=1))
            qT = persist.tile([128, NQ], BF16, tag="qT")
            kT = persist.tile([128, NK], BF16, tag="kT")
            vtok = persist.tile([128, 2, 25, HD], BF16, tag="vtok")
            aoT = persist.tile([128, NQ], BF16, tag="aoT")
            bias_sb = persist.tile([HD, 6], F32, tag="bias")
            nc.sync.dma_start(bias_sb[:], bias6[:])
            ones_sb = persist.tile([HD, HD], BF16, tag="ones")
            nc.sync.dma_start(ones_sb[:], ones2d[:])
            rot_sb = persist.tile([HD, HD], BF16, tag="rot")
            nc.sync.dma_start(rot_sb[:], rotm[:])
            eye_sb = persist.tile([HD, HD], BF16, tag="eye")
            nc.sync.dma_start(eye_sb[:], eye[:])
            ones_f = persist.tile([HD, HD], F32, tag="onesf")
            nc.sync.dma_start(ones_f[:], ones2df[:])
            eye_f = persist.tile([HD, HD], F32, tag="eyef")
            nc.sync.dma_start(eye_f[:], eyef[:])
            wo_sb = [persist.tile([HD, D], BF16, tag=f"wo{u}", name=f"wo_sb{u}")
                     for u in range(2)]
            for u in range(2):
                nc.sync.dma_start(wo_sb[u][:], wo2[u, :, :])
            rsk_sb = persist.tile([128, 50], F32, tag="rsk")
            inv_sb = persist.tile([128, NCHUNK], F32, tag="inv")
            bias_q = persist.tile([128, 1], F32, tag="bias_q")
            nc.vector.memset(bias_q[:], HD * EPS)
            bias_k = persist.tile([128, 1], F32, tag="bias_k")
            nc.vector.memset(bias_k[:], EPS)

            # shared small pools live across rope + attention + out-proj
            row_ps = stack.enter_context(
                tc.tile_pool(name="row_ps", bufs=2, space="PSUM"))   # [1,512]
            smt_ps = stack.enter_context(
                tc.tile_pool(name="smt_ps", bufs=1, space="PSUM"))   # [128,8]
            rtmp = stack.enter_context(tc.tile_pool(name="rtmp", bufs=3))
            cs_pool = stack.enter_context(tc.tile_pool(name="cs", bufs=3))
            pools = {}

            def rms_rs(kind, src_row_psum, w, dst_f32, dst_col0, uid):
                """Transpose ssq row [1,w] to partitions, rsqrt there.

                src_row_psum: [1, w] fp32 PSUM (sum of squares per token).
                Writes 1/sqrt(...) into dst_f32[:, dst_col0 : dst_col0+ncc]
                (f32, token t of 128-chunk i on partition t). Returns ncc.
                """
                ssq_row = rtmp.tile([1, RW], F32, tag="ssqr", name=f"sr{uid}")
                nc.scalar.activation(ssq_row[:, :w], src_row_psum, AF.Copy)
                smt = smt_ps.tile([128, 8], F32, tag="smt", name=f"smt{uid}")
                ncc = (w + 127) // 128
                for i in range(ncc):
                    lo = i * 128
                    ccw = min(128, w - lo)
                    nc.tensor.matmul(
                        smt[0:ccw, 2 * i:2 * i + 2],
                        ssq_row[0:1, lo:lo + ccw],
                        ones_f[0:1, 0:2],
                        start=True, stop=True,
                    )
                sq_t = rtmp.tile([128, 8], F32, tag="sqt", name=f"sq{uid}")
                if kind == "q":
                    nc.scalar.activation(
                        sq_t[:, :ncc], smt[:, 0:2 * ncc:2], AF.Sqrt,
                        bias=bias_q[:], scale=1.0,
                    )
                else:
                    nc.scalar.activation(
                        sq_t[:, :ncc], smt[:, 0:2 * ncc:2], AF.Sqrt,
                        bias=bias_k[:], scale=1.0 / HD,
                    )
                nc.vector.reciprocal(
                    dst_f32[:, dst_col0:dst_col0 + ncc], sq_t[:, :ncc])
                return ncc

            # ---------------- projection phase ----------------
            vT_pool = tc.tile_pool(name="vt", bufs=1)
            with vT_pool as vt_pool:
                vT = vt_pool.tile([128, NK], BF16, tag="vT")
                with tc.tile_pool(name="xt", bufs=1) as xt_pool, \
                     tc.tile_pool(name="wld", bufs=4) as w_pool, \
                     tc.tile_pool(name="pp", bufs=4, space="PSUM") as pp:
                    for half in range(2):
                        h0 = half * 1600
                        xts = []
                        for c in range(NCH):
                            xt = xt_pool.tile([128, 1600], BF16, tag=f"xt{c}")
                            nc.sync.dma_start(
                                xt[:], xT[c * 128:(c + 1) * 128, h0:h0 + 1600])
                            xts.append(xt)
                        # blocks: 0 q_own, 1 q_sh, 2 k_own, 3 k_sh, 4 v_own, 5 v_sh
                        for b in range(6):
                            if b == 1 and half == 1:
                                continue  # shared-head q only needs tokens [0,1600)
                            if b == 0:
                                dst, d0 = qT, h0
                            elif b == 1:
                                dst, d0 = qT, N + h0
                            elif b in (2, 3):
                                dst, d0 = kT, (b - 2) * N + h0
                            else:
                                dst, d0 = vT, (b - 4) * N + h0
                            wtiles = []
                            for c in range(NCH):
                                wt = w_pool.tile([128, HD], BF16, tag="w")
                                nc.sync.dma_start(
                                    wt[:], w6[b, c * 128:(c + 1) * 128, :])
                                wtiles.append(wt)
                            for (o, w) in subtiles(1600, PW):
                                ps = pp.tile([128, PW], F32, tag="pp")
                                for c in range(NCH):
                                    nc.tensor.matmul(
                                        ps[:, :w], wtiles[c][:],
                                        xts[c][:, o:o + w],
                                        start=(c == 0), stop=(c == NCH - 1),
                                    )
                                nc.vector.tensor_scalar_add(
                                    dst[:, d0 + o:d0 + o + w], ps[:, :w],
                                    bias_sb[:, b:b + 1],
                                )

                # ------------- V transpose + rope(k), interleaved -------------
                k_tiles = []
                for seg in range(2):
                    for (ol, w) in subtiles(N, RW):
                        k_tiles.append((seg * N + ol, ol, w))

                with tc.tile_pool(name="ps512", bufs=3, space="PSUM") as ps512:

                    def rope_tile(kind, o, tok, w, uid):
                        big = qT if kind == "q" else kT
                        cdr, sdr = (cq, sq) if kind == "q" else (ck, sk)
                        ct = cs_pool.tile([128, RW], BF16, tag="c", name=f"c{uid}")
                        st = cs_pool.tile([128, RW], BF16, tag="s", name=f"s{uid}")
                        nc.sync.dma_start(ct[:, :w], cdr[:, tok:tok + w])
                        nc.sync.dma_start(st[:, :w], sdr[:, tok:tok + w])
                        src = big[:, o:o + w]
                        q2 = rtmp.tile([128, RW], BF16, tag="q2", name=f"q2{uid}")
                        nc.vector.tensor_mul(q2[:, :w], src, src)
                        ssq = row_ps.tile([1, RW], F32, tag="row", name=f"ssq{uid}")
                        nc.tensor.matmul(
                            ssq[:, :w], ones_sb[:, 0:1], q2[:, :w],
                            start=True, stop=True,
                        )
                        if kind == "k":
                            rms_rs("k", ssq[:, :w], w, rsk_sb, o // 128, uid)
                        else:
                            rsq = rtmp.tile([128, 8], F32, tag="rsq",
                                            name=f"rsq{uid}")
                            ncc = rms_rs("q", ssq[:, :w], w, rsq, 0, uid)
                            # transpose rs back to rows, broadcast via K=1 matmul
                            trp = pools["trp"].tile([128, 128], F32, tag="trp",
                                                    name=f"trp{uid}")
                            nc.tensor.transpose(
                                trp[0:ncc, :], rsq[:, 0:ncc], eye_f[:])
                            # flatten [ncc,128] rows into one sbuf row
                            # (partition-outer order == token order)
                            rows8 = rtmp.tile([8, 128], F32, tag="rows8",
                                              name=f"rows8{uid}")
                            nc.vector.tensor_copy(rows8[0:ncc, :], trp[0:ncc, :])
                            rows = rtmp.tile([1, RW], F32, tag="rows",
                                             name=f"rows{uid}")
                            nc.sync.dma_start(
                                rows[0:1, 0:ncc * 128], rows8[0:ncc, :])
                            bcp = ps512.tile([128, RW], F32, tag="ps",
                                             name=f"bc{uid}")
                            for i in range(ncc):
                                lo = i * 128
                                ccw = min(128, w - lo)
                                nc.tensor.matmul(
                                    bcp[:, lo:lo + ccw], ones_f[0:1, :],
                                    rows[0:1, lo:lo + ccw],
                                    start=True, stop=True,
                                )
                        rot = ps512.tile([128, RW], F32, tag="ps", name=f"rt{uid}")
                        nc.tensor.matmul(
                            rot[:, :w], rot_sb[:], src, start=True, stop=True)
                        m1 = rtmp.tile([128, RW], BF16, tag="m1", name=f"m1{uid}")
                        nc.vector.tensor_mul(m1[:, :w], src, ct[:, :w])
                        m2 = rtmp.tile([128, RW], BF16, tag="m2", name=f"m2{uid}")
                        nc.vector.tensor_mul(m2[:, :w], rot[:, :w], st[:, :w])
                        if kind == "k":
                            nc.vector.tensor_add(src, m1[:, :w], m2[:, :w])
                        else:
                            qr = rtmp.tile([128, RW], BF16, tag="qr",
                                           name=f"qr{uid}")
                            nc.vector.tensor_add(qr[:, :w], m1[:, :w], m2[:, :w])
                            nc.vector.tensor_mul(src, qr[:, :w], bcp[:, :w])

                    # interleave V transposes (5 per psum bank, one ACT copy
                    # per group) with rope(k) so PE, DVE and ACT all stay busy
                    with tc.tile_pool(name="vtp", bufs=2, space="PSUM") as vtp:
                        for i in range(len(k_tiles)):
                            if i < 10:
                                h, g = divmod(i, 5)
                                tpg = vtp.tile([128, 5, HD], BF16, tag="tp",
                                               name=f"tp{i}")
                                for k5 in range(5):
                                    jt = g * 5 + k5
                                    nc.tensor.transpose(
                                        tpg[:, k5, :],
                                        vT[:, h * N + jt * 128:
                                           h * N + (jt + 1) * 128],
                                        eye_sb[:],
                                    )
                                nc.scalar.activation(
                                    vtok[:, h, g * 5:(g + 1) * 5, :], tpg[:],
                                    AF.Copy)
                            (o, tok, w) = k_tiles[i]
                            rope_tile("k", o, tok, w, f"k{i}")

                    # ---------- rope(q) + attention + out-proj, interleaved ----
                    chunks = []
                    for (ol, w) in subtiles(N, RW):
                        chunks.append((0, ol, ol, w))        # unit, q0+co, tok, w
                    for (ol, w) in subtiles(NS, RW):
                        chunks.append((1, N + ol, ol, w))

                    with tc.tile_pool(name="aops", bufs=1, space="PSUM") as aops, \
                         tc.tile_pool(name="trp_ps", bufs=1, space="PSUM") as trp_ps, \
                         tc.tile_pool(name="expp", bufs=3) as expp, \
                         tc.tile_pool(name="yout", bufs=3) as yout:
                        pools["trp"] = trp_ps
                        for ci, (unit, gco, tok, cw) in enumerate(chunks):
                            rope_tile("q", gco, tok, cw, f"q{ci}")

                            head = unit
                            ao = aops.tile([128, RW], F32, tag="ao",
                                           name=f"ao{ci}")
                            sm = row_ps.tile([1, RW], F32, tag="row",
                                             name=f"sm{ci}")
                            prev = None
                            for jt in range(25):
                                gjt = head * 25 + jt
                                sc = ps512.tile([128, RW], F32, tag="ps",
                                                name=f"sc{ci}_{jt}")
                                nc.tensor.matmul(
                                    sc[:, :cw], kT[:, gjt * 128:(gjt + 1) * 128],
                                    qT[:, gco:gco + cw],
                                    start=True, stop=True,
                                )
                                ex = expp.tile([128, RW], BF16, tag="ex",
                                               name=f"ex{ci}_{jt}")
                                nc.scalar.activation(
                                    ex[:, :cw], sc[:, :cw], AF.Exp,
                                    scale=rsk_sb[:, gjt:gjt + 1],
                                )
                                if prev is not None:
                                    pex, pjt = prev
                                    nc.tensor.matmul(
                                        sm[:, :cw], ones_sb[:, 0:1], pex[:, :cw],
                                        start=(pjt == 0), stop=False,
                                    )
                                    nc.tensor.matmul(
                                        ao[:, :cw], vtok[:, head, pjt, :],
                                        pex[:, :cw],
                                        start=(pjt == 0), stop=False,
                                    )
                                prev = (ex, jt)
                            pex, pjt = prev
                            nc.tensor.matmul(
                                sm[:, :cw], ones_sb[:, 0:1], pex[:, :cw],
                                start=False, stop=True,
                            )
                            nc.tensor.matmul(
                                ao[:, :cw], vtok[:, head, pjt, :], pex[:, :cw],
                                start=False, stop=True,
                            )
                            # chunk tail: sums -> partitions -> 1/x; ao -> aoT
                            g0 = gco // 128
                            smrow = rtmp.tile([1, RW], F32, tag="ssqr",
                                              name=f"smr{ci}")
                            nc.scalar.activation(smrow[:, :cw], sm[:, :cw],
                                                 AF.Copy)
                            smt = smt_ps.tile([128, 8], F32, tag="smt",
                                              name=f"smT{ci}")
                            ncc = (cw + 127) // 128
                            for i in range(ncc):
                                lo = i * 128
                                ccw = min(128, cw - lo)
                                nc.tensor.matmul(
                                    smt[0:ccw, 2 * i:2 * i + 2],
                                    smrow[0:1, lo:lo + ccw],
                                    ones_f[0:1, 0:2],
                                    start=True, stop=True,
                                )
                            nc.vector.reciprocal(
                                inv_sb[:, g0:g0 + ncc], smt[:, 0:2 * ncc:2])
                            nc.vector.tensor_copy(
                                aoT[:, gco:gco + cw], ao[:, :cw])

                            # out-projection for this chunk (unnormalized aoT;
                            # per-partition inv scale folded into psum->sbuf copy)
                            ydst = y_own if unit == 0 else y_sh
                            for (it, iw) in subtiles(cw, 128):
                                git = gco + it
                                gidx = git // 128
                                yt = yout.tile([128, D], F32, tag="yt",
                                               name=f"yt{ci}_{it}")
                                for ct3 in range(3):
                                    op = ps512.tile([128, RW], F32, tag="ps",
                                                    name=f"op{ci}_{it}_{ct3}")
                                    nc.tensor.matmul(
                                        op[0:iw, :], aoT[:, git:git + iw],
                                        wo_sb[unit][:, ct3 * 512:(ct3 + 1) * 512],
                                        start=True, stop=True,
                                    )
                                    if ct3 % 2 == 0:
                                        nc.scalar.activation(
                                            yt[0:iw, ct3 * 512:(ct3 + 1) * 512],
                                            op[0:iw, :], AF.Copy,
                                            scale=inv_sb[0:iw, gidx:gidx + 1],
                                        )
                                    else:
                                        nc.vector.tensor_scalar_mul(
                                            yt[0:iw, ct3 * 512:(ct3 + 1) * 512],
                                            op[0:iw, :],
                                            inv_sb[0:iw, gidx:gidx + 1],
                                        )
                                nc.sync.dma_start(
                                    ydst[tok + it:tok + it + iw, :], yt[0:iw, :])

    nc.compile()
    return nc


def _get_nc():
    if "nc" not in _CACHE:
        _CACHE["nc"] = _build()
    return _CACHE["nc"]


def _host_prep(inputs):
    import ml_dtypes

    bf16 = ml_dtypes.bfloat16
    x = np.asarray(inputs["x"], np.float32)[0]          # [N, D]
    Wq = np.asarray(inputs["Wq"], np.float32)
    Wk = np.asarray(inputs["Wk"], np.float32)
    Wv = np.asarray(inputs["Wv"], np.float32)
    Wo = np.asarray(inputs["Wo"], np.float32)
    bq = np.asarray(inputs["bq"], np.float32)
    bk = np.asarray(inputs["bk"], np.float32)
    bv = np.asarray(inputs["bv"], np.float32)
    qs = np.asarray(inputs["q_scale"], np.float32)
    ks = np.asarray(inputs["k_scale"], np.float32)
    ft = np.asarray(inputs["freqs_t"], np.float32)
    fh = np.asarray(inputs["freqs_h"], np.float32)
    fw = np.asarray(inputs["freqs_w"], np.float32)

    cos = np.zeros((N, HD // 2), np.float32)
    sin = np.zeros((N, HD // 2), np.float32)
    idx = np.arange(N)
    f_idx, h_idx, w_idx = idx // (Hg * Wg), (idx // Wg) % Hg, idx % Wg
    cos[:, 0:22], sin[:, 0:22] = ft[f_idx, :, 0], ft[f_idx, :, 1]
    cos[:, 22:43], sin[:, 22:43] = fh[h_idx, :, 0], fh[h_idx, :, 1]
    cos[:, 43:64], sin[:, 43:64] = fw[w_idx, :, 0], fw[w_idx, :, 1]
    C = np.repeat(cos, 2, axis=1).T.copy()               # [128, N]
    S = np.repeat(sin, 2, axis=1).T.copy()
    qs_sw = qs.reshape(64, 2)[:, ::-1].reshape(128)
    ks_sw = ks.reshape(64, 2)[:, ::-1].reshape(128)
    Cq, Sq = C * qs[:, None], S * qs_sw[:, None]
    Ck, Sk = C * ks[:, None], S * ks_sw[:, None]

    rotm = np.zeros((128, 128), np.float32)
    pr = np.arange(64)
    rotm[2 * pr + 1, 2 * pr] = -1.0
    rotm[2 * pr, 2 * pr + 1] = 1.0
    eye = np.eye(128, dtype=np.float32)
    ones2d = np.ones((128, 128), np.float32)

    xT = np.ascontiguousarray(x.T)                       # [D, N]
    perm_swap = np.concatenate([np.arange(1600, N), np.arange(0, 1600)])

    in_maps = []
    for core in range(8):
        pair, parity = core // 2, core % 2
        own, sh = 3 * pair + parity, 3 * pair + 2
        if parity == 0:
            xTc, Cqc, Sqc, Ckc, Skc = xT, Cq, Sq, Ck, Sk
        else:
            xTc = np.ascontiguousarray(xT[:, perm_swap])
            Cqc = np.ascontiguousarray(Cq[:, perm_swap])
            Sqc = np.ascontiguousarray(Sq[:, perm_swap])
            Ckc = np.ascontiguousarray(Ck[:, perm_swap])
            Skc = np.ascontiguousarray(Sk[:, perm_swap])
        w6 = np.stack([
            Wq[:, own * HD:(own + 1) * HD], Wq[:, sh * HD:(sh + 1) * HD],
            Wk[:, own * HD:(own + 1) * HD], Wk[:, sh * HD:(sh + 1) * HD],
            Wv[:, own * HD:(own + 1) * HD], Wv[:, sh * HD:(sh + 1) * HD],
        ])
        bias6 = np.stack([
            bq[own * HD:(own + 1) * HD], bq[sh * HD:(sh + 1) * HD],
            bk[own * HD:(own + 1) * HD], bk[sh * HD:(sh + 1) * HD],
            bv[own * HD:(own + 1) * HD], bv[sh * HD:(sh + 1) * HD],
        ], axis=1)
        wo2 = np.stack([
            Wo[own * HD:(own + 1) * HD, :], Wo[sh * HD:(sh + 1) * HD, :],
        ])
        in_maps.append({
            "xT": xTc.astype(bf16), "w6": np.ascontiguousarray(w6).astype(bf16),
            "bias6": np.ascontiguousarray(bias6),
            "cq": Cqc.astype(bf16), "sq": Sqc.astype(bf16),
            "ck": Ckc.astype(bf16), "sk": Skc.astype(bf16),
            "rotm": rotm.astype(bf16), "eye": eye.astype(bf16),
            "ones2d": ones2d.astype(bf16), "ones2df": ones2d, "eyef": eye,
            "wo2": np.ascontiguousarray(wo2).astype(bf16),
        })
    return in_maps, perm_swap


def _gather(results, perm_swap, bo):
    inv_swap = perm_swap  # swapping halves is its own inverse
    y = np.zeros((N, D), np.float32)
    for core in range(8):
        parity = core % 2
        yo = np.asarray(results[core]["y_own"], np.float32)
        ysh = np.asarray(results[core]["y_sh"], np.float32)
        if parity == 0:
            y += yo
            y[0:1600] += ysh
        else:
            y += yo[inv_swap]
            y[1600:3200] += ysh
    y += bo[None, :]
    return y[None]


def run_internal(inputs, trace=False, **kw):
    from concourse.bass_utils import run_bass_kernel_spmd

    nc = _get_nc()
    in_maps, perm_swap = _host_prep(inputs)
    res = run_bass_kernel_spmd(
        nc, in_maps, core_ids=list(range(8)), trace=trace, **kw
    )
    bo = np.asarray(inputs["bo"], np.float32)
    y = _gather(res.results, perm_swap, bo)
    return y, res


def kernel(**inputs):
    y, _ = run_internal(inputs, trace=False)
    return y
